# revision 1
# baseline (speedup 1.0000x reference)
"""Trainium2 Bass kernel for nn_DenseFeatureExtractionModule_AP_OS4.

Sharding: 8 cores = 4 images x 2 vertical halves. Bottom halves are flipped
vertically on host (conv weights' dy flipped too) so every core runs the same
"top-mode" program: input slab rows [0:240) of a (possibly flipped) image,
output rows [0:48) at stride-4 resolution.

All matmuls run in float32r (full-rate on TRN2 PE, ~1e-4 relative rounding).
conv1 is im2col K=27; conv2/conv3 pack tap pairs into K=128 using a
column-shifted stacked copy on partitions 64..127. gconv8..10 build the
spatially-varying-dilation rhs with copy_predicated selection per tap, so
only one conv's worth of matmuls is needed.
"""
import sys

if '/opt/trn_rl_repo' not in sys.path:
    sys.path.insert(0, '/opt/trn_rl_repo')

import numpy as np

import concourse.tile as tile
from concourse import bacc, mybir
from concourse.bass_utils import run_bass_kernel_spmd

F32 = mybir.dt.float32
F32R = mybir.dt.float32r
U8 = mybir.dt.uint8
AF = mybir.ActivationFunctionType

TAPS = [(dy, dx) for dy in (-1, 0, 1) for dx in (-1, 0, 1)]

# geometry (top-mode, per core)
XROWS = 240          # input slab rows at 384 res
C1_ROWS = 233        # conv1 out rows
C2_ROWS = 232        # conv2 out rows
P1_ROWS = 116        # pool1 out rows (192 res)
C3_ROWS = 115
C4_ROWS = 114
P2_ROWS = 57         # pool2 out rows (96 res)
C5_ROWS = 56
C6_ROWS = 55
C7_ROWS = 54
PS_ROWS = 54         # irregular-pool/select out rows
G8_ROWS = 52
G9_ROWS = 50
G10_ROWS = 48
W384, W192, W96 = 384, 192, 96

BAND_A = 8           # conv2-out rows per stage-A band (29 bands)
BAND_B = 12          # conv4-out rows per stage-B band

BOFF = {1: 0, 2: 1, 3: 2, 4: 3, 5: 4, 6: 6, 7: 8, 8: 10, 9: 14, 10: 18}


def split_rows(total, pref=5):
    """Split `total` rows into tiles of size <=pref, all >=3 (so N>=288)."""
    if total <= pref:
        return [total]
    sizes = [pref] * (total // pref)
    rem = total - pref * len(sizes)
    if rem == 1:
        sizes[-1] = 3
        sizes.append(3)
    elif rem == 2:
        sizes[-1] = 4
        sizes.append(3)
    elif rem > 0:
        sizes.append(rem)
    assert sum(sizes) == total
    return sizes


import os
UPTO = os.environ.get("KERNEL_UPTO", "")


def build_program():
    nc = bacc.Bacc("TRN2", target_bir_lowering=False, debug=False,
                   enable_asserts=True, num_devices=8)
    dram = lambda name, shape, dt=F32, kind="ExternalInput": \
        nc.dram_tensor(name, list(shape), dt, kind=kind).ap()

    T = {}
    T["x_in"] = dram("x_in", [3, XROWS, W384])
    T["mu8"] = dram("mu8", [128, 56, W96], U8)
    T["mf32"] = dram("mf32", [128, 56, W96])
    T["w1"] = dram("w1", [27, 128])
    T["w2p"] = dram("w2p", [3, 128, 64])
    T["w2s"] = dram("w2s", [3, 64, 64])
    T["w3p"] = dram("w3p", [3, 128, 128])
    T["w3s"] = dram("w3s", [3, 64, 128])
    T["w4"] = dram("w4", [9, 128, 128])
    T["w5"] = dram("w5", [9, 128, 256])
    T["w6"] = dram("w6", [9, 256, 256])
    T["w7"] = dram("w7", [9, 256, 256])
    T["w8"] = dram("w8", [9, 256, 512])
    T["w9"] = dram("w9", [9, 512, 512])
    T["w10"] = dram("w10", [9, 512, 512])
    T["ball"] = dram("ball", [128, 22])
    T["y"] = dram("y", [512, G10_ROWS, W96], kind="ExternalOutput")

    with tile.TileContext(nc) as tc:
        _emit(nc, tc, T, UPTO)
    nc.compile()
    return nc


def _emit(nc, tc, T, upto=""):
    from contextlib import ExitStack
    ctx = ExitStack()
    with ctx:
        # whole-kernel pools
        wsm = ctx.enter_context(tc.tile_pool(name="wsm", bufs=1))
        dramp = ctx.enter_context(tc.tile_pool(name="dramp", bufs=1, space="DRAM"))

        bt = wsm.tile([128, 22], F32, name="bt")
        nc.sync.dma_start(bt[:], T["ball"][:])

        def bias(l, m=0, parts=128):
            return bt[0:parts, BOFF[l] + m:BOFF[l] + m + 1]

        # ---- stage A/B small weights ----
        w1t = wsm.tile([27, 128], F32R, name="w1t")
        nc.sync.dma_start(w1t[:], T["w1"][:].bitcast(F32R))
        w2pt = wsm.tile([128, 3, 64], F32R, name="w2pt")
        w2st = wsm.tile([64, 3, 64], F32R, name="w2st")
        w3pt = wsm.tile([128, 3, 128], F32R, name="w3pt")
        w3st = wsm.tile([64, 3, 128], F32R, name="w3st")
        for i in range(3):
            nc.sync.dma_start(w2pt[:, i, :], T["w2p"][i].bitcast(F32R))
            nc.sync.dma_start(w2st[:, i, :], T["w2s"][i].bitcast(F32R))
            nc.sync.dma_start(w3pt[:, i, :], T["w3p"][i].bitcast(F32R))
            nc.sync.dma_start(w3st[:, i, :], T["w3s"][i].bitcast(F32R))
        zt = wsm.tile([128, 2 * W384], F32, name="zt")
        nc.vector.memset(zt[:], 0.0)
        w4t = wsm.tile([128, 9, 128], F32R, name="w4t")
        for t in range(9):
            nc.sync.dma_start(w4t[:, t, :], T["w4"][t].bitcast(F32R))

        # DRAM intermediates
        x2pd = dramp.tile([64, P1_ROWS, W192], F32, name="x2pd")
        xpd = dramp.tile([256, PS_ROWS, W96], F32, name="xpd")
        x8d = dramp.tile([512, G8_ROWS, W96], F32, name="x8d")
        x9d = dramp.tile([512, G9_ROWS, W96], F32, name="x9d")

        # ================= stage A: conv1 + conv2 + pool1 -> x2pd =========
        with tc.tile_pool(name="stA", bufs=2) as pA, \
             tc.tile_pool(name="psA", bufs=4, space="PSUM") as psA:
            b0 = 0
            while b0 < C2_ROWS:
                c2a, c2b_ = b0, min(b0 + BAND_A, C2_ROWS)
                c2len = c2b_ - c2a
                c1lo = c2a - 1                     # x1e row 0 == conv1 row c1lo
                c1a, c1b = max(c1lo, 0), min(c2b_ + 1, C1_ROWS)
                n1 = c1b - c1a
                xlo = c1a - 1                      # xt row 0 == x row xlo
                xa, xb_ = max(xlo, 0), min(c1b + 1, XROWS)

                xt = pA.tile([3, BAND_A + 4, W384 + 4], F32R, name="xt", tag="xt")
                nc.vector.memset(xt[:, :, 0:2].bitcast(F32), 0.0)
                nc.vector.memset(xt[:, :, 2 + W384:].bitcast(F32), 0.0)
                if xa > xlo:
                    nc.vector.memset(xt[:, 0:xa - xlo, :].bitcast(F32), 0.0)
                nc.sync.dma_start(xt[:, xa - xlo:xb_ - xlo, 2:2 + W384],
                                  T["x_in"][:, xa:xb_, :].bitcast(F32R))
                r27 = pA.tile([27, BAND_A + 2, W384 + 4], F32R, name="r27", tag="r27")
                for ti, (dy, dx) in enumerate(TAPS):
                    src_r0 = (c1a + dy) - xlo
                    nc.sync.dma_start(
                        r27[3 * ti:3 * ti + 3, 0:n1, 2:2 + W384],
                        xt[:, src_r0:src_r0 + n1, 2 + dx:2 + dx + W384])
                x1e = pA.tile([128, BAND_A + 2, W384 + 4], F32R, name="x1e", tag="x1e")
                if c1a > c1lo:
                    nc.vector.memset(x1e[:, 0:c1a - c1lo, :].bitcast(F32), 0.0)
                nc.vector.memset(x1e[:, :, 0:2].bitcast(F32), 0.0)
                nc.vector.memset(x1e[:, :, 2 + W384:].bitcast(F32), 0.0)
                r = 0
                while r < n1:
                    rp = min(2, n1 - r)
                    ps = psA.tile([128, 2, 512], F32, name="psA", tag="psA")
                    for j in range(rp):
                        nc.tensor.matmul(ps[:, j, 0:W384], w1t[:],
                                         r27[:, r + j, 2:2 + W384],
                                         start=True, stop=True)
                    ro = (c1a + r) - c1lo
                    nc.scalar.activation(
                        x1e[0:64, ro:ro + rp, 2:2 + W384],
                        ps[0:64, 0:rp, 0:W384], AF.Relu, bias=bias(1, parts=64))
                    nc.vector.scalar_tensor_tensor(
                        x1e[64:128, ro:ro + rp, 1:1 + W384],
                        ps[64:128, 0:rp, 0:W384],
                        bt[64:128, BOFF[1]:BOFF[1] + 1],
                        zt[64:128, 0:rp * W384].rearrange(
                            "p (a b) -> p a b", a=rp),
                        mybir.AluOpType.add, mybir.AluOpType.max)
                    r += rp
                c2t = pA.tile([64, BAND_A, W384], F32, name="c2t", tag="c2t")
                for r in range(0, c2len, 2):
                    ps = psA.tile([64, 2, 512], F32, name="psA2", tag="psA")
                    for j in range(2):
                        first = True
                        for dyi, dy in enumerate((-1, 0, 1)):
                            ro = (c2a + r + j + dy) - c1lo
                            nc.tensor.matmul(ps[:, j, 0:W384], w2pt[:, dyi, :],
                                             x1e[:, ro, 1:1 + W384],
                                             start=first, stop=False)
                            first = False
                        for dyi, dy in enumerate((-1, 0, 1)):
                            ro = (c2a + r + j + dy) - c1lo
                            nc.tensor.matmul(ps[:, j, 0:W384], w2st[:, dyi, :],
                                             x1e[0:64, ro, 3:3 + W384],
                                             start=False, stop=(dyi == 2))
                    nc.scalar.activation(c2t[:, r:r + 2, :], ps[:, :, 0:W384],
                                         AF.Relu, bias=bias(2, parts=64))
                ph = pA.tile([64, BAND_A, W192], F32, name="phA", tag="phA")
                nc.vector.tensor_max(ph[:, 0:c2len, :], c2t[:, 0:c2len, 0:W384:2],
                                     c2t[:, 0:c2len, 1:W384:2])
                plen = c2len // 2
                pb = pA.tile([64, BAND_A // 2, W192], F32, name="pb", tag="pb")
                nc.vector.tensor_max(pb[:, 0:plen, :],
                                     ph[:, 0:c2len:2, :], ph[:, 1:c2len:2, :])
                nc.sync.dma_start(x2pd[:, c2a // 2:c2a // 2 + plen, :],
                                  pb[:, 0:plen, :])
                b0 = c2b_

        psp = ctx.enter_context(tc.tile_pool(name="psp", bufs=8, space="PSUM"))
        if upto == "A":
            return
        # ============ stage B..irregular-pool: shared resident ring =======
        with tc.tile_pool(name="ring", bufs=2) as ringp:
            x3p = ringp.tile([128, P2_ROWS + 4, W96 + 4], F32R,
                             name="x3p", tag="ring")
            nc.vector.memset(x3p[:].bitcast(F32), 0.0)
            with tc.tile_pool(name="stB", bufs=2) as pB:
                b0 = 0
                while b0 < C4_ROWS:
                    c4a, c4b_ = b0, min(b0 + BAND_B, C4_ROWS)
                    c4len = c4b_ - c4a
                    c3lo = c4a - 1
                    c3a, c3b = max(c3lo, 0), min(c4b_ + 1, C3_ROWS)
                    n3 = c3b - c3a
                    xlo = c4a - 2
                    xa, xb_ = max(xlo, 0), min(c4b_ + 2, P1_ROWS)
                    x2b = pB.tile([128, BAND_B + 4, W192 + 4], F32R,
                                  name="x2b", tag="x2b")
                    nc.vector.memset(x2b[:, :, 0:2].bitcast(F32), 0.0)
                    nc.vector.memset(x2b[:, :, 193:].bitcast(F32), 0.0)
                    if xa > xlo:
                        nc.vector.memset(x2b[:, 0:xa - xlo, :].bitcast(F32), 0.0)
                    nc.sync.dma_start(x2b[0:64, xa - xlo:xb_ - xlo, 2:2 + W192],
                                      x2pd[:, xa:xb_, :].bitcast(F32R))
                    nc.sync.dma_start(x2b[64:128, xa - xlo:xb_ - xlo, 1:1 + W192],
                                      x2pd[:, xa:xb_, :].bitcast(F32R))
                    c3t = pB.tile([128, BAND_B + 2, W192 + 4], F32R,
                                  name="c3t", tag="c3t")
                    if c3a > c3lo:
                        nc.vector.memset(c3t[:, 0:c3a - c3lo, :].bitcast(F32), 0.0)
                    nc.vector.memset(c3t[:, :, 0:2].bitcast(F32), 0.0)
                    nc.vector.memset(c3t[:, :, 2 + W192:].bitcast(F32), 0.0)
                    r = 0
                    while r < n3:
                        rt = 2 if r + 2 <= n3 else 1
                        g0 = c3a + r
                        ps = psp.tile([128, 2, W192], F32, name="psB", tag="ps")
                        first = True
                        for dyi, dy in enumerate((-1, 0, 1)):
                            ro = (g0 + dy) - xlo
                            nc.tensor.matmul(ps[:, 0:rt, :], w3pt[:, dyi, :],
                                             x2b[:, ro:ro + rt, 1:1 + W192],
                                             start=first, stop=False)
                            first = False
                        for dyi, dy in enumerate((-1, 0, 1)):
                            ro = (g0 + dy) - xlo
                            nc.tensor.matmul(ps[:, 0:rt, :], w3st[:, dyi, :],
                                             x2b[0:64, ro:ro + rt, 3:3 + W192],
                                             start=False, stop=(dyi == 2))
                        nc.scalar.activation(
                            c3t[:, (g0 - c3lo):(g0 - c3lo) + rt, 2:2 + W192],
                            ps[:, 0:rt, :], AF.Relu, bias=bias(3))
                        r += rt
                    c4t = pB.tile([128, BAND_B, W192], F32, name="c4t", tag="c4t")
                    r = 0
                    while r < c4len:
                        rt = 2
                        ps = psp.tile([128, 2, W192], F32, name="psB2", tag="ps")
                        for ti, (dy, dx) in enumerate(TAPS):
                            ro = (c4a + r + dy) - c3lo
                            nc.tensor.matmul(ps[:, 0:rt, :], w4t[:, ti, :],
                                             c3t[:, ro:ro + rt, 2 + dx:2 + dx + W192],
                                             start=(ti == 0), stop=(ti == 8))
                        nc.scalar.activation(c4t[:, r:r + rt, :], ps[:, 0:rt, :],
                                             AF.Relu, bias=bias(4))
                        r += rt
                    ph = pB.tile([128, BAND_B, W96], F32, name="phB", tag="phB")
                    nc.vector.tensor_max(ph[:, 0:c4len, :],
                                         c4t[:, 0:c4len, 0:W192:2],
                                         c4t[:, 0:c4len, 1:W192:2])
                    p0 = c4a // 2
                    plen = c4len // 2
                    nc.vector.tensor_max(x3p[:, 2 + p0:2 + p0 + plen, 2:2 + W96],
                                         ph[:, 0:c4len:2, :], ph[:, 1:c4len:2, :])
                    b0 = c4b_

            if upto == "B":
                return
            # ---------------- conv5/6/7 ----------------
            def dense_conv(view_fn, G_in, rows_out, w_tiles, Mt, lay, out_writer):
                r0 = 0
                for rt in split_rows(rows_out):
                    pss = [psp.tile([128, 5, W96], F32, name=f"c{lay}ps{m}",
                                    tag="ps") for m in range(Mt)]
                    nmm = G_in * 9
                    i = 0
                    for k in range(G_in):
                        for ti, (dy, dx) in enumerate(TAPS):
                            rhs = view_fn(k, 2 + r0 + dy, rt, 2 + dx)
                            for m in range(Mt):
                                nc.tensor.matmul(
                                    pss[m][:, 0:rt, :],
                                    w_tiles[k][:, ti, 128 * m:128 * (m + 1)],
                                    rhs, start=(i == 0), stop=(i == nmm - 1))
                            i += 1
                    for m in range(Mt):
                        out_writer(pss[m], m, r0, rt)
                    r0 += rt

            with tc.tile_pool(name="wk256", bufs=4) as wk256:
                def load_w256(w_ap, Kt, lay):
                    tiles = []
                    for k in range(Kt):
                        wk = wk256.tile([128, 9, 256], F32R,
                                        name=f"w{lay}k{k}", tag="wk256")
                        for t in range(9):
                            nc.sync.dma_start(
                                wk[:, t, :],
                                w_ap[t, 128 * k:128 * (k + 1), :].bitcast(F32R))
                        tiles.append(wk)
                    return tiles

                x5 = ringp.tile([128, 2, C5_ROWS + 4, W96 + 4], F32R,
                                name="x5", tag="ring")
                nc.vector.memset(x5[:].bitcast(F32), 0.0)
                w5t = load_w256(T["w5"], 1, 5)

                def wr5(ps, m, r0, rt):
                    nc.scalar.activation(x5[:, m, 2 + r0:2 + r0 + rt, 2:2 + W96],
                                         ps[:, 0:rt, :], AF.Relu, bias=bias(5, m))
                dense_conv(lambda k, rs, rt, cs: x3p[:, rs:rs + rt, cs:cs + W96],
                           1, C5_ROWS, w5t, 2, 5, wr5)

                x6 = ringp.tile([128, 2, C6_ROWS + 4, W96 + 4], F32R,
                                name="x6", tag="ring")
                nc.vector.memset(x6[:].bitcast(F32), 0.0)
                w6t = load_w256(T["w6"], 2, 6)

                def wr6(ps, m, r0, rt):
                    nc.scalar.activation(x6[:, m, 2 + r0:2 + r0 + rt, 2:2 + W96],
                                         ps[:, 0:rt, :], AF.Relu, bias=bias(6, m))
                dense_conv(lambda k, rs, rt, cs: x5[:, k, rs:rs + rt, cs:cs + W96],
                           2, C6_ROWS, w6t, 2, 6, wr6)

                x7 = ringp.tile([128, 2, C7_ROWS + 4, W96 + 4], F32R,
                                name="x7", tag="ring")
                nc.vector.memset(x7[:].bitcast(F32), 0.0)
                w7t = load_w256(T["w7"], 2, 7)

                def wr7(ps, m, r0, rt):
                    nc.scalar.activation(x7[:, m, 2 + r0:2 + r0 + rt, 2:2 + W96],
                                         ps[:, 0:rt, :], AF.Relu, bias=bias(7, m))
                dense_conv(lambda k, rs, rt, cs: x6[:, k, rs:rs + rt, cs:cs + W96],
                           2, C7_ROWS, w7t, 2, 7, wr7)

            if upto == "C7":
                return
            # ---- irregular pool + select -> xpd (DRAM) ----
            mu = wsm.tile([128, 56, W96], U8, name="mu")
            nc.sync.dma_start(mu[:], T["mu8"][:])
            with tc.tile_pool(name="irp", bufs=1) as irp, \
                 tc.tile_pool(name="irq", bufs=2) as irq:
                mf = irp.tile([128, 56, W96], F32, name="mf")
                nc.sync.dma_start(mf[:], T["mf32"][:])
                for g in range(2):
                    phg = irq.tile([128, PS_ROWS, W96 // 2], F32,
                                   name="phg", tag="phg")
                    nc.vector.tensor_max(
                        phg[:],
                        x7[:, g, 2:2 + PS_ROWS, 2:2 + W96:2].bitcast(F32),
                        x7[:, g, 2:2 + PS_ROWS, 3:3 + W96:2].bitcast(F32))
                    pv = irq.tile([128, PS_ROWS // 2, W96 // 2], F32,
                                  name="pv", tag="pv")
                    nc.vector.tensor_max(pv[:], phg[:, 0:PS_ROWS:2, :],
                                         phg[:, 1:PS_ROWS:2, :])
                    irb = irq.tile([128, PS_ROWS, W96], F32, name="irb", tag="irb",
                                   bufs=1)
                    for a in (0, 1):
                        for bq in (0, 1):
                            tq = irq.tile([128, PS_ROWS // 2, W96 // 2], F32,
                                          name="tq", tag="tq")
                            nc.vector.tensor_mul(
                                tq[:], pv[:], mf[:, a:PS_ROWS:2, bq:W96:2])
                            nc.vector.tensor_max(
                                irb[:, a:PS_ROWS:2, bq:W96:2],
                                x7[:, g, 2 + a:2 + PS_ROWS:2,
                                   2 + bq:2 + W96:2].bitcast(F32),
                                tq[:])
                    nc.sync.dma_start(xpd[128 * g:128 * (g + 1), :, :], irb[:])

        # ================= gconv layers (stream from DRAM) ================
        evp = ctx.enter_context(tc.tile_pool(name="evp", bufs=8))
        selp = ctx.enter_context(tc.tile_pool(name="selp", bufs=8))
        gbnd = ctx.enter_context(tc.tile_pool(name="gbnd", bufs=3))
        wk512 = ctx.enter_context(tc.tile_pool(name="wk512", bufs=4))

        def load_w512(w_ap, Kt, lay):
            tiles = []
            for k in range(Kt):
                wk = wk512.tile([128, 9, 512], F32R, name=f"w{lay}k{k}",
                                tag="wk512")
                for t in range(9):
                    nc.sync.dma_start(
                        wk[:, t, :],
                        w_ap[t, 128 * k:128 * (k + 1), :].bitcast(F32R))
                tiles.append(wk)
            return tiles

        def sel_rhs(v1, v2, rt, r0):
            if os.environ.get("KERNEL_NOSEL"):
                return v1
            tmp = selp.tile([128, 5, W96], F32, name="selt", tag="selt")
            nc.sync.dma_start(tmp[:, 0:rt, :], v1.bitcast(F32))
            nc.vector.copy_predicated(tmp[:, 0:rt, :], mu[:, r0:r0 + rt, :],
                                      v2.bitcast(F32))
            rhs = selp.tile([128, 5, W96], F32R, name="selr", tag="selr")
            nc.scalar.activation(rhs[:, 0:rt, :], tmp[:, 0:rt, :], AF.Copy)
            return rhs
        sel_rhs.flip = False

        def gconv(xsrc, in_rows, Kt, rows_out, w_tiles, Mt, lay, out_writer):
            sizes = split_rows(rows_out)
            groups = []
            gi = 0
            r0 = 0
            while gi < len(sizes):
                grp = [(r0, sizes[gi])]
                r0 += sizes[gi]
                if not os.environ.get("KERNEL_NOPAIR") and gi + 1 < len(sizes):
                    grp.append((r0, sizes[gi + 1]))
                    r0 += sizes[gi + 1]
                    gi += 2
                else:
                    gi += 1
                groups.append(grp)
            for grp in groups:
                bbs = []
                for (t0, rt) in grp:
                    bb = gbnd.tile([128, 4, 9, W96 + 4], F32R, name="bb", tag="bb")
                    nc.gpsimd.memset(bb[:].bitcast(F32), 0.0)
                    lo, hi = t0 - 2, t0 + rt + 2
                    a, b = max(lo, 0), min(hi, in_rows)
                    for k in range(Kt):
                        nc.sync.dma_start(
                            bb[:, k, a - lo:b - lo, 2:2 + W96],
                            xsrc[:][128 * k:128 * (k + 1), a:b, :].bitcast(F32R))
                    bbs.append(bb)
                pss = [[psp.tile([128, 5, W96], F32, name=f"g{lay}ps{m}_{n}",
                                 tag="ps") for m in range(Mt)]
                       for n in range(len(grp))]
                nmm = Kt * 9
                i = 0
                for k in range(Kt):
                    for ti, (dy, dx) in enumerate(TAPS):
                        rhss = []
                        for n, (t0, rt) in enumerate(grp):
                            bb = bbs[n]
                            v1 = bb[:, k, 2 + dy:2 + dy + rt,
                                    2 + dx:2 + dx + W96]
                            v2 = bb[:, k, 2 + 2 * dy:2 + 2 * dy + rt,
                                    2 + 2 * dx:2 + 2 * dx + W96]
                            rhs = sel_rhs(v1, v2, rt, t0)
                            rhss.append(
                                rhs if os.environ.get("KERNEL_NOSEL")
                                else rhs[:, 0:rt, :])
                        for m in range(Mt):
                            for n, (t0, rt) in enumerate(grp):
                                nc.tensor.matmul(
                                    pss[n][m][:, 0:rt, :],
                                    w_tiles[k][:, ti, 128 * m:128 * (m + 1)],
                                    rhss[n],
                                    start=(i == 0), stop=(i == nmm - 1))
                        i += 1
                for n, (t0, rt) in enumerate(grp):
                    for m in range(Mt):
                        out_writer(pss[n][m], m, t0, rt)

        def make_writer(lay, dst, rows):
            def wr(ps, m, r0, rt):
                rv = min(rt, rows - r0)
                if rv <= 0:
                    return
                ev = evp.tile([128, 5, W96], F32, name=f"ev{lay}", tag="ev")
                nc.scalar.activation(ev[:, 0:rv, :], ps[:, 0:rv, :], AF.Relu,
                                     bias=bias(lay, m))
                nc.sync.dma_start(dst[128 * m:128 * (m + 1), r0:r0 + rv, :],
                                  ev[:, 0:rv, :])
            return wr

        w8t = load_w512(T["w8"], 2, 8)
        gconv(xpd, PS_ROWS, 2, G8_ROWS, w8t, 4, 8,
              make_writer(8, x8d[:], G8_ROWS))
        if upto == "G8":
            return
        w9t = load_w512(T["w9"], 4, 9)
        gconv(x8d, G8_ROWS, 4, G9_ROWS, w9t, 4, 9,
              make_writer(9, x9d[:], G9_ROWS))
        if upto == "G9":
            return
        w10t = load_w512(T["w10"], 4, 10)
        gconv(x9d, G9_ROWS, 4, G10_ROWS, w10t, 4, 10,
              make_writer(10, T["y"], G10_ROWS))


# ---------------- host side ----------------
_NC_CACHE = None


def get_program():
    global _NC_CACHE
    if _NC_CACHE is None:
        _NC_CACHE = build_program()
    return _NC_CACHE


def _prep_taps(W):
    """[cout, cin, 3, 3] -> [9, cin, cout], tap index = (dy+1)*3 + (dx+1)."""
    return np.ascontiguousarray(
        W.transpose(2, 3, 1, 0).reshape(9, W.shape[1], W.shape[0])
    ).astype(np.float32)


def _pack_pairs(wt):
    """wt [9, cin, cout] -> pairs [(dy,-1)+(dy,0)] K=2*cin, singles [(dy,+1)]."""
    cin, cout = wt.shape[1], wt.shape[2]
    pairs = np.zeros((3, 2 * cin, cout), np.float32)
    singles = np.zeros((3, cin, cout), np.float32)
    for dyi in range(3):
        pairs[dyi, 0:cin] = wt[dyi * 3 + 0]       # dx = -1
        pairs[dyi, cin:2 * cin] = wt[dyi * 3 + 1]  # dx = 0
        singles[dyi] = wt[dyi * 3 + 2]             # dx = +1
    return pairs, singles


def prepare_in_maps(batch, pooling_mask, Ws, bs):
    ball = np.zeros((128, 22), np.float32)
    ball[64:128, BOFF[1]] = np.asarray(bs[1], np.float32)  # upper conv1 copy
    for l in range(1, 11):
        b = np.asarray(bs[l], np.float32)
        Mt = max(1, b.shape[0] // 128)
        for m in range(Mt):
            seg = b[m * 128:(m + 1) * 128]
            ball[0:seg.shape[0], BOFF[l] + m] = seg
    in_maps = []
    for core in range(8):
        img, h = core // 2, core % 2
        x = np.asarray(batch[img], np.float32)
        m48 = (np.asarray(pooling_mask[img, 0]) > 0)
        if h == 1:
            x = x[:, ::-1, :]
            m48 = m48[::-1, :]
        x = np.ascontiguousarray(x[:, 0:XROWS, :])
        mup = np.repeat(np.repeat(m48, 2, axis=0), 2, axis=1)[0:56, :]
        mu8 = np.ascontiguousarray(
            np.broadcast_to(mup.astype(np.uint8)[None], (128, 56, 96)))
        mf32 = np.ascontiguousarray(
            np.broadcast_to(mup.astype(np.float32)[None], (128, 56, 96)))
        wd = {}
        for l in range(1, 11):
            W = np.asarray(Ws[l], np.float32)
            if h == 1:
                W = W[:, :, ::-1, :]
            wd[l] = _prep_taps(W)
        w2pair, w2sing = _pack_pairs(wd[2])
        w3pair, w3sing = _pack_pairs(wd[3])
        in_maps.append({
            "x_in": x, "mu8": mu8, "mf32": mf32, "ball": ball,
            "w1": np.ascontiguousarray(
                np.concatenate([wd[1].reshape(27, 64)] * 2, axis=1)),
            "w2p": w2pair, "w2s": w2sing,
            "w3p": w3pair, "w3s": w3sing,
            "w4": wd[4], "w5": wd[5], "w6": wd[6], "w7": wd[7],
            "w8": wd[8], "w9": wd[9], "w10": wd[10],
        })
    return in_maps


def assemble_output(results):
    out = np.zeros((4, 512, 96, 96), np.float32)
    for core in range(8):
        img, h = core // 2, core % 2
        y = results[core]["y"]
        if h == 0:
            out[img, :, 0:48, :] = y
        else:
            out[img, :, 48:96, :] = y[:, ::-1, :]
    return out


def kernel(**inputs):
    batch = np.asarray(inputs["batch"], np.float32)
    pooling_mask = np.asarray(inputs["pooling_mask"])
    Ws = {l: np.asarray(inputs[f"W{l}"], np.float32) for l in range(1, 11)}
    bs = {l: np.asarray(inputs[f"b{l}"], np.float32) for l in range(1, 11)}
    nc = get_program()
    in_maps = prepare_in_maps(batch, pooling_mask, Ws, bs)
    res = run_bass_kernel_spmd(nc, in_maps, core_ids=list(range(8)))
    return assemble_output(res.results)

